# revision 1
# baseline (speedup 1.0000x reference)
"""MultiHeadAttention (cosine-sim, no softmax) + residual + LayerNorm on 8 TRN2 cores.

Reference math (per sample row x of q/k/v, D=2048, H=16, HD=128):
  qp = q @ Wq + bq   (kept as residual)
  kp = k @ Wk + bk ; vp = v @ Wv + bv
  per head h: qn = qh/||qh||, kn = kh/||kh||
  s[h,g] = (qn_h . kn_g) / HD          # [16,16] per sample
  o[h] = sum_g s[h,g] * vh_g           # [16,128]
  o_flat[hd*16+h] = o[h,hd]            # interleaved flatten
  o2 = o_flat @ Wo + bo
  x = qp + o2 ; out = layernorm(x) * gamma + beta

Sharding: pure data-parallel over batch (4096 rows/core), weights replicated.

Device strategy (per core), all heavy matmuls in bf16 with fp32 PSUM accum:
  - Host pre-transposes q,k,v -> [D, B] bf16 so activation blocks load as
    [k,b] tiles directly (PE contracts over the partition dim).
  - qp, kp computed in natural space: psum[128b, 512n] += qT_blk.T @ W[k,n512].
  - vp computed transposed: psum[128n, 512b] += Wv[k,n128].T @ vT[k,b512],
    giving vhT[hd, g, b] directly (head g = row-block g of vpT).
  - Per-head normalize of qp/kp in natural space (free-dim reduce), then PE
    transposes to qnT/knT[hd, h, b].
  - Scores for 8 samples at once: ST[(g,s),(h,s')] = knT_s8.T @ qnT_s8; the
    block-diagonal (s==s') entries are the real scores; multiply by a host
    constant mask (1/HD on diag blocks, 0 elsewhere) -> masked bf16 ST.
  - o for 8 samples in one matmul: oT[hd,(h,s)] = V_blk.T @ ST_masked where
    V_blk[(g,s),hd] is a PE transpose of a vhT slice. Cross-sample terms
    vanish because ST_masked is block-diagonal.
  - Output projection uses host-permuted Wo' (row hd*16+h -> h*128+hd) so
    o2[128b,512n] += oT[:,h,bt].T @ Wo'[h*128:,n512] accumulated over h.
  - Residual add + layernorm fused on-chip; fp32 output.
"""

from contextlib import ExitStack

import numpy as np
import ml_dtypes

import concourse.bass as bass
import concourse.bacc as bacc
import concourse.mybir as mybir
import concourse.tile as tile
from concourse.bass_utils import run_bass_kernel_spmd

BF16 = mybir.dt.bfloat16
F32 = mybir.dt.float32

B, D, H, HD = 32768, 2048, 16, 128
NCORES = 8
EPS = 1e-5
CHUNK = 512          # samples per chunk (8 chunks per core at BL=4096)
NG = D // 512        # 4 n-chunks of 512 columns
KO = D // 128        # 16 contraction blocks
SBLK = 8             # samples per attention block matmul (8*16 = 128)


def build_bass(bl, repeat=1):
    """Build the per-core Bass program for bl batch rows (bl % CHUNK == 0)."""
    nc = bacc.Bacc()

    qT = nc.dram_tensor("qT", [D, bl], BF16, kind="ExternalInput")
    kT = nc.dram_tensor("kT", [D, bl], BF16, kind="ExternalInput")
    vT = nc.dram_tensor("vT", [D, bl], BF16, kind="ExternalInput")
    Wq = nc.dram_tensor("Wq", [D, D], BF16, kind="ExternalInput")
    Wk = nc.dram_tensor("Wk", [D, D], BF16, kind="ExternalInput")
    Wv = nc.dram_tensor("Wv", [D, D], BF16, kind="ExternalInput")
    Wo = nc.dram_tensor("Wo", [D, D], BF16, kind="ExternalInput")  # permuted on host
    bq = nc.dram_tensor("bq", [1, D], BF16, kind="ExternalInput")
    bk = nc.dram_tensor("bk", [1, D], BF16, kind="ExternalInput")
    bo = nc.dram_tensor("bo", [1, D], BF16, kind="ExternalInput")
    bv = nc.dram_tensor("bv", [128, KO], F32, kind="ExternalInput")  # [p, nb] layout
    gamma = nc.dram_tensor("gamma", [1, D], F32, kind="ExternalInput")
    beta = nc.dram_tensor("beta", [1, D], F32, kind="ExternalInput")
    ident = nc.dram_tensor("ident", [128, 128], BF16, kind="ExternalInput")
    mask = nc.dram_tensor("mask", [128, 128], BF16, kind="ExternalInput")
    ones = nc.dram_tensor("ones", [1, 128], BF16, kind="ExternalInput")
    out = nc.dram_tensor("out", [bl, D], F32, kind="ExternalOutput")

    nchunks = bl // CHUNK
    NBT = CHUNK // 128  # b-tiles per chunk

    with tile.TileContext(nc) as tc, ExitStack() as ctx:
        consts = ctx.enter_context(tc.tile_pool(name="consts", bufs=1))
        qkvT_pool = ctx.enter_context(tc.tile_pool(name="qkvT", bufs=1))
        wko_pool = ctx.enter_context(tc.tile_pool(name="wko", bufs=8))
        chunk_pool = ctx.enter_context(tc.tile_pool(name="chunkbuf", bufs=1))
        trans_pool = ctx.enter_context(tc.tile_pool(name="trans", bufs=3))
        small_pool = ctx.enter_context(tc.tile_pool(name="small", bufs=4))
        out_pool = ctx.enter_context(tc.tile_pool(name="outb", bufs=3))
        proj_psum = ctx.enter_context(tc.tile_pool(name="proj_psum", bufs=4, space="PSUM"))
        att_psum = ctx.enter_context(tc.tile_pool(name="att_psum", bufs=4, space="PSUM"))

        # ---- constants ----
        ident_sb = consts.tile([128, 128], BF16)
        nc.sync.dma_start(out=ident_sb, in_=ident[:, :])
        mask_sb = consts.tile([128, 128], BF16)
        nc.sync.dma_start(out=mask_sb, in_=mask[:, :])
        ones_sb = consts.tile([1, 128], BF16)
        nc.sync.dma_start(out=ones_sb, in_=ones[:, :])
        bq_sb = consts.tile([1, D], BF16)
        nc.sync.dma_start(out=bq_sb, in_=bq[:, :])
        bk_sb = consts.tile([1, D], BF16)
        nc.sync.dma_start(out=bk_sb, in_=bk[:, :])
        bo_sb = consts.tile([1, D], BF16)
        nc.sync.dma_start(out=bo_sb, in_=bo[:, :])
        bv_sb = consts.tile([128, KO], F32)
        nc.sync.dma_start(out=bv_sb, in_=bv[:, :])
        eps_sb = consts.tile([128, 1], F32)
        nc.vector.memset(eps_sb, EPS)
        # gamma/beta broadcast across all 128 partitions (DMA partition step 0)
        g_ap = gamma[:, :]
        gamma_sb = consts.tile([128, D], F32)
        nc.sync.dma_start(
            out=gamma_sb,
            in_=bass.AP(tensor=g_ap.tensor, offset=g_ap.offset,
                        ap=[[0, 128], [1, D]]),
        )
        b_ap = beta[:, :]
        beta_sb = consts.tile([128, D], F32)
        nc.sync.dma_start(
            out=beta_sb,
            in_=bass.AP(tensor=b_ap.tensor, offset=b_ap.offset,
                        ap=[[0, 128], [1, D]]),
        )

        qT3 = qT.rearrange("(ko p) b -> p ko b", p=128)
        kT3 = kT.rearrange("(ko p) b -> p ko b", p=128)
        vT3 = vT.rearrange("(ko p) b -> p ko b", p=128)
        Wq3 = Wq.rearrange("(ko p) n -> p ko n", p=128)
        Wk3 = Wk.rearrange("(ko p) n -> p ko n", p=128)
        Wv3 = Wv.rearrange("(ko p) n -> p ko n", p=128)
        Wo3 = Wo.rearrange("(ko p) n -> p ko n", p=128)

        for _rep in range(repeat):
          for c in range(nchunks):
            b0 = c * CHUNK
            # chunk-resident activation inputs [128, KO, CHUNK] bf16
            qT_sb = qkvT_pool.tile([128, KO, CHUNK], BF16, tag="qT")
            nc.sync.dma_start(out=qT_sb, in_=qT3[:, :, b0:b0 + CHUNK])
            kT_sb = qkvT_pool.tile([128, KO, CHUNK], BF16, tag="kT")
            nc.sync.dma_start(out=kT_sb, in_=kT3[:, :, b0:b0 + CHUNK])
            vT_sb = qkvT_pool.tile([128, KO, CHUNK], BF16, tag="vT")
            nc.sync.dma_start(out=vT_sb, in_=vT3[:, :, b0:b0 + CHUNK])

            # chunk-lifetime buffers
            # qnT/knT/vhT use interleaved [hd, blk, h, s] layout (b = blk*8+s)
            # so a per-block slice [:, blk] is one contiguous 128-wide free dim
            # as required by matmul operands.
            NBLK = CHUNK // SBLK
            qp_sb = chunk_pool.tile([128, NBT, D], F32, tag="qp")      # residual (becomes x)
            qnT_sb = chunk_pool.tile([128, NBLK, H, SBLK], BF16, tag="qnT")
            knT_sb = chunk_pool.tile([128, NBLK, H, SBLK], BF16, tag="knT")
            vhT_sb = chunk_pool.tile([128, NBLK, H, SBLK], BF16, tag="vhT")
            oT_sb = chunk_pool.tile([128, H, CHUNK], BF16, tag="oT")

            # ---- q,k projections (natural space) + normalize + transpose ----
            for ng in range(NG):
                n0 = ng * 512
                for (xT_sb, W3, b_sb, is_q) in (
                    (qT_sb, Wq3, bq_sb, True),
                    (kT_sb, Wk3, bk_sb, False),
                ):
                    ps_list = [proj_psum.tile([128, 512], F32, tag="pp",
                                               name=f"pp_{c}_{ng}_{is_q}_{bt}")
                               for bt in range(NBT)]
                    for ko in range(KO):
                        w_sb = wko_pool.tile([128, 512], BF16, tag="w")
                        nc.scalar.dma_start(out=w_sb, in_=W3[:, ko, n0:n0 + 512])
                        for bt in range(NBT):
                            nc.tensor.matmul(
                                ps_list[bt],
                                xT_sb[:, ko, bt * 128:(bt + 1) * 128],
                                w_sb, start=(ko == 0), stop=False)
                    for bt in range(NBT):
                        ps = ps_list[bt]
                        # bias via K=1 ones-row matmul (broadcast along partitions)
                        nc.tensor.matmul(ps, ones_sb, b_sb[:, n0:n0 + 512],
                                         start=False, stop=True)
                        if is_q:
                            # keep fp32 residual
                            nc.scalar.copy(out=qp_sb[:, bt, n0:n0 + 512], in_=ps)
                        # per-head 1/||.|| for the 4 heads in this n-chunk
                        # (single-input ACT Square + accumulate; DVE can't
                        # read the same PSUM tile twice)
                        rr = small_pool.tile([128, 4], F32, tag="rr")
                        scratch = trans_pool.tile([128, 128], BF16, tag="scr")
                        for h4 in range(4):
                            nc.scalar.activation(
                                out=scratch,
                                in_=ps[:, h4 * 128:(h4 + 1) * 128],
                                func=mybir.ActivationFunctionType.Square,
                                accum_out=rr[:, h4:h4 + 1])
                        nc.scalar.activation(out=rr, in_=rr,
                                             func=mybir.ActivationFunctionType.Sqrt)
                        nc.vector.reciprocal(out=rr, in_=rr)
                        # normalized bf16 copy (natural layout)
                        nrm = trans_pool.tile([128, 512], BF16, tag="nrm")
                        for h4 in range(4):
                            nc.vector.tensor_scalar_mul(
                                out=nrm[:, h4 * 128:(h4 + 1) * 128],
                                in0=ps[:, h4 * 128:(h4 + 1) * 128],
                                scalar1=rr[:, h4:h4 + 1])
                        # transpose each head block -> [hd, b], scatter into
                        # interleaved [hd, blk, h, s] layout
                        dstT = qnT_sb if is_q else knT_sb
                        for h4 in range(4):
                            tp = att_psum.tile([128, 128], BF16, tag="ap")
                            nc.tensor.transpose(
                                tp, nrm[:, h4 * 128:(h4 + 1) * 128], ident_sb)
                            nc.scalar.copy(
                                out=dstT[:, bt * 16:(bt + 1) * 16, ng * 4 + h4, :],
                                in_=tp[:, :].rearrange(
                                    "p (blk s) -> p blk s", s=SBLK))

            # ---- v projection (transposed space) -> vhT ----
            for g in range(H):
                ps_v = [proj_psum.tile([128, 256], F32, tag="pp",
                                        name=f"pv_{c}_{g}_{half}")
                        for half in range(CHUNK // 256)]
                for ko in range(KO):
                    w_sb = wko_pool.tile([128, 128], BF16, tag="wv")
                    nc.scalar.dma_start(out=w_sb,
                                        in_=Wv3[:, ko, g * 128:(g + 1) * 128])
                    for half in range(CHUNK // 256):
                        nc.tensor.matmul(
                            ps_v[half], w_sb,
                            vT_sb[:, ko, half * 256:(half + 1) * 256],
                            start=(ko == 0), stop=(ko == KO - 1))
                for half in range(CHUNK // 256):
                    # add per-partition bias while copying psum->sbuf bf16
                    nc.scalar.activation(
                        out=vhT_sb[:, half * 32:(half + 1) * 32, g, :],
                        in_=ps_v[half][:, :].rearrange(
                            "p (blk s) -> p blk s", s=SBLK),
                        func=mybir.ActivationFunctionType.Identity,
                        bias=bv_sb[:, g:g + 1], scale=1.0)

            # ---- attention: scores + o, 8 samples per matmul ----
            for blk in range(CHUNK // SBLK):
                s0 = blk * SBLK
                # ST[(g,s),(h,s')] = knT_s8.T @ qnT_s8   (contraction over hd)
                st_ps = att_psum.tile([128, 128], F32, tag="ap")
                nc.tensor.matmul(
                    st_ps,
                    knT_sb[:, blk].rearrange("p h s -> p (h s)"),
                    qnT_sb[:, blk].rearrange("p h s -> p (h s)"),
                    start=True, stop=True)
                st_sb = trans_pool.tile([128, 128], BF16, tag="st")
                nc.vector.tensor_mul(out=st_sb, in0=st_ps, in1=mask_sb)
                # V_blk[(g,s),hd] = transpose(vhT[:, blk])
                vb_ps = att_psum.tile([128, 128], BF16, tag="ap")
                nc.tensor.transpose(
                    vb_ps, vhT_sb[:, blk].rearrange("p h s -> p (h s)"), ident_sb)
                vb_sb = trans_pool.tile([128, 128], BF16, tag="vb")
                nc.scalar.copy(out=vb_sb, in_=vb_ps)
                # oT[hd,(h,s)] = V_blk.T @ ST_masked
                o_ps = att_psum.tile([128, 128], F32, tag="ap")
                nc.tensor.matmul(o_ps, vb_sb, st_sb, start=True, stop=True)
                nc.scalar.copy(
                    out=oT_sb[:, :, s0:s0 + SBLK],
                    in_=o_ps[:, :].rearrange("p (h s) -> p h s", h=H))

            # ---- output projection + residual + layernorm ----
            for ng in range(NG):
                n0 = ng * 512
                ps_list = [proj_psum.tile([128, 512], F32, tag="pp",
                                           name=f"po_{c}_{ng}_{bt}")
                           for bt in range(NBT)]
                for h in range(H):
                    w_sb = wko_pool.tile([128, 512], BF16, tag="w")
                    nc.scalar.dma_start(out=w_sb, in_=Wo3[:, h, n0:n0 + 512])
                    for bt in range(NBT):
                        nc.tensor.matmul(
                            ps_list[bt], oT_sb[:, h, bt * 128:(bt + 1) * 128],
                            w_sb, start=(h == 0), stop=False)
                for bt in range(NBT):
                    ps = ps_list[bt]
                    nc.tensor.matmul(ps, ones_sb, bo_sb[:, n0:n0 + 512],
                                     start=False, stop=True)
                    # x = qp + o2 (in place into qp_sb)
                    nc.vector.tensor_add(
                        out=qp_sb[:, bt, n0:n0 + 512],
                        in0=qp_sb[:, bt, n0:n0 + 512], in1=ps)

            for bt in range(NBT):
                x_ap = qp_sb[:, bt, :]
                stats = small_pool.tile([128, 4, 6], F32, tag="bn")
                for sg in range(4):
                    nc.vector.bn_stats(out=stats[:, sg, :],
                                       in_=x_ap[:, sg * 512:(sg + 1) * 512])
                mv = small_pool.tile([128, 2], F32, tag="mv")
                nc.vector.bn_aggr(out=mv, in_=stats)
                rstd = small_pool.tile([128, 1], F32, tag="rstd")
                nc.scalar.activation(out=rstd, in_=mv[:, 1:2],
                                     func=mybir.ActivationFunctionType.Sqrt,
                                     bias=eps_sb, scale=1.0)
                nc.vector.reciprocal(out=rstd, in_=rstd)
                for ng in range(NG):
                    n0 = ng * 512
                    ot = out_pool.tile([128, 512], F32, tag="ot")
                    # (x - mu) * rstd
                    nc.vector.tensor_scalar(
                        out=ot, in0=x_ap[:, n0:n0 + 512],
                        scalar1=mv[:, 0:1], scalar2=rstd,
                        op0=mybir.AluOpType.subtract,
                        op1=mybir.AluOpType.mult)
                    # * gamma
                    nc.vector.tensor_mul(out=ot, in0=ot,
                                         in1=gamma_sb[:, n0:n0 + 512])
                    # + beta
                    nc.gpsimd.tensor_add(out=ot, in0=ot,
                                         in1=beta_sb[:, n0:n0 + 512])
                    nc.sync.dma_start(
                        out=out[b0 + bt * 128:b0 + (bt + 1) * 128, n0:n0 + 512],
                        in_=ot)

    nc.compile()
    return nc


def _prep_host_inputs(q, k, v, Wq, bq, Wk, bk, Wv, bv, Wo, bo, gamma, beta):
    bf = ml_dtypes.bfloat16
    qT = np.ascontiguousarray(q.T).astype(bf)
    kT = np.ascontiguousarray(k.T).astype(bf)
    vT = np.ascontiguousarray(v.T).astype(bf)
    # Wo' row h*128+hd  <- Wo row hd*16+h
    hh, dd = np.divmod(np.arange(D), HD)     # d' = h*HD+hd -> h=hh, hd=dd
    src = dd * H + hh
    Wo_p = np.ascontiguousarray(Wo[src, :]).astype(bf)
    # block-diag mask, 1/HD on (r,c) where r%8 == c%8
    r = np.arange(128)
    m = (r[:, None] % SBLK == r[None, :] % SBLK).astype(np.float32) / HD
    shared = {
        "Wq": np.ascontiguousarray(Wq).astype(bf),
        "Wk": np.ascontiguousarray(Wk).astype(bf),
        "Wv": np.ascontiguousarray(Wv).astype(bf),
        "Wo": Wo_p,
        "bq": bq.reshape(1, D).astype(bf),
        "bk": bk.reshape(1, D).astype(bf),
        "bo": bo.reshape(1, D).astype(bf),
        "bv": np.ascontiguousarray(bv.reshape(KO, 128).T).astype(np.float32),
        "gamma": gamma.reshape(1, D).astype(np.float32),
        "beta": beta.reshape(1, D).astype(np.float32),
        "ident": np.eye(128, dtype=bf),
        "mask": m.astype(bf),
        "ones": np.ones((1, 128), dtype=bf),
    }
    return qT, kT, vT, shared


def kernel(q, k, v, Wq, bq, Wk, bk, Wv, bv, Wo, bo, gamma, beta, _bl=None,
           _ncores=None, _trace=False):
    ncores = _ncores or NCORES
    bl = _bl or (q.shape[0] // ncores)
    qT, kT, vT, shared = _prep_host_inputs(
        q, k, v, Wq, bq, Wk, bk, Wv, bv, Wo, bo, gamma, beta)
    nc = build_bass(bl)
    in_maps = []
    for c in range(ncores):
        m = dict(shared)
        s = slice(c * bl, (c + 1) * bl)
        m["qT"] = np.ascontiguousarray(qT[:, s])
        m["kT"] = np.ascontiguousarray(kT[:, s])
        m["vT"] = np.ascontiguousarray(vT[:, s])
        in_maps.append(m)
    res = run_bass_kernel_spmd(nc, in_maps, core_ids=list(range(ncores)),
                               trace=_trace)
    outs = [r["out"] for r in res.results]
    full = np.concatenate(outs, axis=0)
    if _trace:
        kernel.last_results = res
    return full.astype(np.float32)

